# revision 20
# baseline (speedup 1.0000x reference)
"""Trainium2 Bass kernel for nn_CompactControlAttention.

The module's attention is degenerate: softmax over a size-1 axis is exactly
1.0, so queries/keys (Wq, bq, Wk, bk) never affect the output:

    out[b, s, :] = sequence[b, s, :] + p[b, :]
    p = (sum_c controls[c]) @ Wv.T @ Wo.T + C * (bv @ Wo.T + bo)

Sharding: tensor-parallel over the hidden feature dim f of v = cs @ Wv.T
and over the output feature dim e of p. Cross-core exchange of the tiny
v.T (256KB) happens between two NEFF launches via host gather -- on-chip
collectives cost ~75us of fixed setup per execute on this stack, and HBM
is only pair-shared, so a host hop is the cheapest 8-way exchange.

NEFF-1 (per core k, ~3MB DMA):
  cs_t = sum_c controls_t[c]      (controls shipped pre-transposed, bf16)
  v_k  = cs @ Wv.T[:, fk] + C*bv  (16 bf16 matmuls, 256-wide, PSUM accum)
  vt_k = v_k.T                    (2 PE transposes) -> out [256, 64] bf16

NEFF-2 (per core k, ~4.3MB DMA; host feeds the gathered full v.T):
  p_k  = v @ Wo.T[:, ek] + bo     (16 bf16 matmuls)
  out  = seq_k + broadcast_s(p_k) (chunked DVE/GpSimd adds, piped DMA)
"""

import numpy as np
import ml_dtypes

import concourse.bass as bass
import concourse.mybir as mybir
import concourse.tile as tile
from concourse import bacc
from concourse.bass_utils import run_bass_kernel_spmd
from concourse.masks import make_identity

N_CORES = 8
D = 2048
B = 64
S = 32
C = 8
EK = D // N_CORES  # 256
NT = D // 128  # 16
F32 = mybir.dt.float32
BF16 = mybir.dt.bfloat16

_CACHE = {}


# --------------------------- NEFF-1: v.T slice ---------------------------


def _build_nc1():
    nc = bacc.Bacc("TRN2", target_bir_lowering=False, debug=False, num_devices=N_CORES)
    ctrl = nc.dram_tensor("ctrl", [D, C * B], BF16, kind="ExternalInput")
    wvt = nc.dram_tensor("wvt", [D, EK], BF16, kind="ExternalInput")  # Wv.T[:, fk]
    bv = nc.dram_tensor("bv", [EK], F32, kind="ExternalInput")
    vt_out = nc.dram_tensor("vt", [EK, B], BF16, kind="ExternalOutput")

    with tile.TileContext(nc) as tc:
        from contextlib import ExitStack

        ctx = ExitStack()
        P = 128
        consts = ctx.enter_context(tc.tile_pool(name="consts", bufs=1))
        sbuf = ctx.enter_context(tc.tile_pool(name="sbuf", bufs=1))
        psum_v = ctx.enter_context(tc.tile_pool(name="psum_v", bufs=1, space="PSUM"))
        psum_t = ctx.enter_context(tc.tile_pool(name="psum_t", bufs=1, space="PSUM"))

        # ctrl in eighths (4 per HWDGE queue) so cs adds pipeline with DMA
        ctrl_sb = sbuf.tile([P, NT * C * B], BF16)
        c3 = ctrl_sb[:].rearrange("p (t cb) -> p t cb", cb=C * B)
        Q = NT // 8  # 2 t-tiles per piece
        for qi in range(8):
            q = nc.sync if qi % 2 == 0 else nc.scalar
            q.dma_start(
                out=c3[:, qi * Q : (qi + 1) * Q, :],
                in_=ctrl[qi * 256 : (qi + 1) * 256, :].rearrange(
                    "(t p) cb -> p t cb", p=P
                ),
            )
        wv_sb = sbuf.tile([P, NT * EK], BF16)
        wv4 = wv_sb[:].rearrange("p (q t f) -> p q (t f)", q=4, f=EK)
        for qi in range(4):
            q = nc.sync if qi % 2 == 0 else nc.scalar
            q.dma_start(
                out=wv4[:, qi, :].rearrange("p (t f) -> p t f", f=EK),
                in_=wvt[qi * 512 : (qi + 1) * 512, :].rearrange(
                    "(t p) f -> p t f", p=P
                ),
            )
        bv_sb = consts.tile([1, EK], F32)
        nc.gpsimd.dma_start(out=bv_sb[:], in_=bv[None, :])

        ident = consts.tile([P, P], F32)
        make_identity(nc, ident[:])
        ident_b = consts.tile([P, P], BF16)
        nc.vector.tensor_copy(ident_b[:], ident[:])
        ones8_f = consts.tile([1, B], F32)
        nc.vector.memset(ones8_f[:], float(C))
        ones8 = consts.tile([1, B], BF16)
        nc.vector.tensor_copy(ones8[:], ones8_f[:])
        bv_b = consts.tile([1, EK], BF16)
        nc.vector.tensor_copy(bv_b[:], bv_sb[:])

        # cs tree sum, per ctrl eighth as it lands (DVE + GpSimd split)
        c4 = ctrl_sb[:].rearrange("p (t c b) -> p t c b", c=C, b=B)
        s1 = sbuf.tile([P, NT * 4 * B], BF16)
        s1v = s1[:].rearrange("p (t c b) -> p t c b", c=4, b=B)
        s2 = sbuf.tile([P, NT * 2 * B], BF16)
        s2v = s2[:].rearrange("p (t c b) -> p t c b", c=2, b=B)
        cs = sbuf.tile([P, NT * B], BF16)
        csv = cs[:].rearrange("p (t b) -> p t b", b=B)
        for qi in range(8):
            ts = slice(qi * Q, (qi + 1) * Q)
            eng = nc.vector if qi % 2 == 0 else nc.gpsimd
            eng.tensor_add(s1v[:, ts], c4[:, ts, 0:4, :], c4[:, ts, 4:8, :])
            eng.tensor_add(s2v[:, ts], s1v[:, ts, 0:2, :], s1v[:, ts, 2:4, :])
            eng.tensor_add(csv[:, ts], s2v[:, ts, 0, :], s2v[:, ts, 1, :])

        # MM1 + bias
        pv = psum_v.tile([B, EK], F32, tag="pv")
        wv3 = wv_sb[:].rearrange("p (t f) -> p t f", f=EK)
        for t in range(NT):
            nc.tensor.matmul(
                pv[:], csv[:, t, :], wv3[:, t, :], start=(t == 0), stop=False
            )
        nc.tensor.matmul(pv[:], ones8[:], bv_b[:], start=False, stop=True)
        v = sbuf.tile([B, EK], BF16)
        nc.vector.tensor_copy(v[:], pv[:])

        # vt = v.T
        pt = psum_t.tile([P, 2 * B], BF16, tag="pt")
        for g in range(2):
            nc.tensor.transpose(
                pt[:, g * B : (g + 1) * B], v[:, g * 128 : (g + 1) * 128],
                ident_b[0:B, 0:B],
            )
        vt = sbuf.tile([P, 2 * B], BF16)
        nc.vector.tensor_copy(vt[:], pt[:])
        nc.sync.dma_start(
            out=vt_out[:].rearrange("(g p) b -> p g b", p=P),
            in_=vt[:].rearrange("p (g b) -> p g b", b=B),
        )
        ctx.close()
    nc.compile()
    return nc


# ------------------------ NEFF-2: MM2 + residual -------------------------


def _build_nc2():
    nc = bacc.Bacc("TRN2", target_bir_lowering=False, debug=False, num_devices=N_CORES)
    vta = nc.dram_tensor("vta", [D, B], BF16, kind="ExternalInput")  # full v.T
    wot = nc.dram_tensor("wot", [D, EK], BF16, kind="ExternalInput")  # Wo.T[:, ek]
    bo = nc.dram_tensor("bo", [EK], F32, kind="ExternalInput")
    seq = nc.dram_tensor("seq", [128, S * 128], BF16, kind="ExternalInput")
    out = nc.dram_tensor("out", [128, S * 128], F32, kind="ExternalOutput")

    with tile.TileContext(nc) as tc:
        from contextlib import ExitStack

        ctx = ExitStack()
        P = 128
        consts = ctx.enter_context(tc.tile_pool(name="consts", bufs=1))
        sbuf = ctx.enter_context(tc.tile_pool(name="sbuf", bufs=1))
        psum_p = ctx.enter_context(tc.tile_pool(name="psum_p", bufs=1, space="PSUM"))

        vta_sb = sbuf.tile([P, NT * B], BF16)
        vta3 = vta_sb[:].rearrange("p (t b) -> p t b", b=B)
        nc.sync.dma_start(out=vta3, in_=vta.rearrange("(t p) b -> p t b", p=P))
        # wot first on BOTH queues (quarters) -- MM2's critical input
        wot_sb = sbuf.tile([P, NT * EK], BF16)
        wo_h = wot_sb[:].rearrange("p (h t e) -> p h (t e)", h=4, e=EK)
        for hi in range(4):
            q = nc.scalar if hi % 2 == 0 else nc.sync
            q.dma_start(
                out=wo_h[:, hi, :].rearrange("p (t e) -> p t e", e=EK),
                in_=wot[hi * 512 : (hi + 1) * 512, :].rearrange(
                    "(t p) e -> p t e", p=P
                ),
            )
        bo_sb = consts.tile([1, EK], F32)
        nc.gpsimd.dma_start(out=bo_sb[:], in_=bo[None, :])
        seq_sb = sbuf.tile([P, S * 128], BF16)
        nc.sync.dma_start(out=seq_sb[:, 0 : S * 64], in_=seq[:, 0 : S * 64])
        nc.scalar.dma_start(out=seq_sb[:, S * 64 :], in_=seq[:, S * 64 :])

        ones1_f = consts.tile([1, B], F32)
        nc.vector.memset(ones1_f[:], 1.0)
        ones1 = consts.tile([1, B], BF16)
        nc.vector.tensor_copy(ones1[:], ones1_f[:])
        bo_b = consts.tile([1, EK], BF16)
        nc.vector.tensor_copy(bo_b[:], bo_sb[:])

        pp = psum_p.tile([B, EK], F32, tag="pp")
        wo3 = wot_sb[:].rearrange("p (t e) -> p t e", e=EK)
        for t in range(NT):
            nc.tensor.matmul(
                pp[:], vta3[:, t, :], wo3[:, t, :], start=(t == 0), stop=False
            )
        nc.tensor.matmul(pp[:], ones1[:], bo_b[:], start=False, stop=True)

        p_re = sbuf.tile([P, P], F32)
        nc.vector.tensor_copy(p_re[0:B, :], pp[:, 0:P])
        nc.vector.tensor_copy(p_re[B : 2 * B, :], pp[:, P : 2 * P])

        out_sb = sbuf.tile([P, S * 128], F32)
        o3 = out_sb[:].rearrange("p (s e) -> p s e", e=P)
        q3 = seq_sb[:].rearrange("p (s e) -> p s e", e=P)
        chunks = [  # (engine, s0, s1, queue)
            (nc.vector, 0, 8, nc.sync),
            (nc.vector, 8, 16, nc.scalar),
            (nc.vector, 16, 24, nc.sync),
            (nc.vector, 24, 32, nc.scalar),
        ]
        for eng, s0, s1, q in chunks:
            eng.tensor_add(
                o3[:, s0:s1, :], q3[:, s0:s1, :],
                p_re[:, None, :].to_broadcast((P, s1 - s0, P)),
            )
            q.dma_start(
                out=out[:, s0 * 128 : s1 * 128], in_=out_sb[:, s0 * 128 : s1 * 128]
            )
        ctx.close()
    nc.compile()
    return nc


def _get_ncs():
    if "nc1" not in _CACHE:
        _CACHE["nc1"] = _build_nc1()
        _CACHE["nc2"] = _build_nc2()
    return _CACHE["nc1"], _CACHE["nc2"]


def _run(inputs, trace=False):
    nc1, nc2 = _get_ncs()
    bf = ml_dtypes.bfloat16
    sequence = np.asarray(inputs["sequence"])
    controls = np.asarray(inputs["controls"])
    Wv = np.asarray(inputs["Wv"])
    bv = np.asarray(inputs["bv"])
    Wo = np.asarray(inputs["Wo"])
    bo = np.asarray(inputs["bo"])

    ctrl_t = np.ascontiguousarray(
        controls.transpose(2, 0, 1).reshape(D, C * B).astype(bf)
    )
    in1 = []
    for k in range(N_CORES):
        fk = slice(k * EK, (k + 1) * EK)
        in1.append(
            {
                "ctrl": ctrl_t,
                "wvt": np.ascontiguousarray(Wv[fk, :].T.astype(bf)),
                "bv": np.ascontiguousarray(bv[fk]),
            }
        )
    res1 = run_bass_kernel_spmd(nc1, in1, list(range(N_CORES)), trace=trace)

    vta = np.concatenate([np.asarray(res1.results[k]["vt"]) for k in range(N_CORES)])
    vta = np.ascontiguousarray(vta)  # (D, B) bf16

    in2 = []
    for k in range(N_CORES):
        ek = slice(k * EK, (k + 1) * EK)
        in2.append(
            {
                "vta": vta,
                "wot": np.ascontiguousarray(Wo[ek, :].T.astype(bf)),
                "bo": np.ascontiguousarray(bo[ek]),
                "seq": np.ascontiguousarray(
                    sequence[:, :, ek]
                    .reshape(B, S, 2, 128)
                    .transpose(2, 0, 1, 3)
                    .reshape(128, S * 128)
                    .astype(bf)
                ),
            }
        )
    res2 = run_bass_kernel_spmd(nc2, in2, list(range(N_CORES)), trace=trace)

    out = np.empty((B, S, D), dtype=np.float32)
    for k in range(N_CORES):
        out[:, :, k * EK : (k + 1) * EK] = (
            res2.results[k]["out"]
            .reshape(2, B, S, 128)
            .transpose(1, 2, 0, 3)
            .reshape(B, S, EK)
        )
    return out, (res1, res2)


def kernel(**inputs):
    out, _ = _run(inputs)
    return out
